# revision 9
# baseline (speedup 1.0000x reference)
"""Trainium2 Bass kernel for BeatDetectionRSNN2 (2-layer integrate-and-fire RSNN).

Reference semantics (per time step t):
    v1 += x_t @ W1.T ; s1 = (v1 >= 1); v1 *= (1 - s1)
    v2 += s1 @ W2.T  ; s2 = (v2 >= 1); v2 *= (1 - s2)
    out[:, t, :] = s2

Sharding: data-parallel over batch across 8 cores (16 batch rows each),
weights replicated, time recurrence local per core.

Per-core plan (all shapes hardcoded: B_c=16, T=4096, D=128, O=2):
  - PE computes u1 = x @ W1.T ahead of time in K-step chunks
    (x tiles [t,d] are PE-transposed to [d,t]; one big matmul per chunk).
  - The serial recurrence runs on the vector engine over a [128, 17] state:
    cols 0..15 = v1 laid out [d, b], col 16 = v2 for the 32 (b,o) pairs on
    partitions 0..31 (layer-2 fused into the same instructions, consuming
    u2 produced 2 chunks behind). The state is split into two independent
    streams (A: batches 0-7; B: batches 8-15 + the v2 column) whose
    dependent instruction pairs are interleaved, hiding most of the
    store-visibility stall between dependent ops on the engine:
        I1: w = v + u          (tensor_tensor add)
        I2: v = (w < 1) * w    (scalar_tensor_tensor)
  - Spikes s1 = (w >= 1) are extracted per chunk (gpsimd) and fed to PE
    for u2 = s1 @ W2.T; s2 comes from col 16 of w (extracted on gpsimd).
"""
import sys
import numpy as np

if '/opt/trn_rl_repo' not in sys.path:
    sys.path.insert(0, '/opt/trn_rl_repo')

import concourse.bacc as bacc
import concourse.tile as tile
import concourse.mybir as mybir
from concourse.masks import make_identity
from concourse.bass_utils import run_bass_kernel_spmd

f32 = mybir.dt.float32
Alu = mybir.AluOpType

B, T, D, O = 128, 4096, 128, 2
NCORES = 8
BC = B // NCORES          # 16 batch rows per core
K = 128                   # chunk (time steps)
NC = T // K               # 32 chunks
FD = BC + 1               # 17 chain columns
HB = 8                    # batches in stream A (stream B: 8 batches + v2 col)


def build_program(t_steps=T):
    nch = t_steps // K
    nc = bacc.Bacc("TRN2", target_bir_lowering=False)
    x_ext = nc.declare_dram_parameter("x", [BC, t_steps, D], f32, isOutput=False)
    w1t_ext = nc.declare_dram_parameter("w1t", [D, D], f32, isOutput=False)
    w2t_ext = nc.declare_dram_parameter("w2t", [D, O], f32, isOutput=False)
    # output stored (o, b, t) so the per-chunk DMA from [32, K] staging
    # (partition p = 16*o + b) is contiguous; host transposes to [b, t, o].
    out_ext = nc.declare_dram_parameter("out", [O, BC, t_steps], f32, isOutput=True)

    with tile.TileContext(nc) as tc:
        with (
            tc.tile_pool(name="consts", bufs=1) as consts,
            tc.tile_pool(name="xin", bufs=6) as xin_pool,
            tc.tile_pool(name="xT", bufs=2) as xT_pool,
            tc.tile_pool(name="ubuf", bufs=4) as u_pool,
            tc.tile_pool(name="wbuf", bufs=3) as w_pool,
            tc.tile_pool(name="gbuf", bufs=2) as g_pool,
            tc.tile_pool(name="s2st", bufs=3) as s2_pool,
            tc.tile_pool(name="u2ep", bufs=2) as u2ep_pool,
            tc.tile_pool(name="u2sb", bufs=2) as u2sb_pool,
            tc.tile_pool(name="u2c", bufs=3) as u2c_pool,
            tc.tile_pool(name="wep", bufs=2) as wep_pool,
            tc.tile_pool(name="xpose", bufs=2, space="PSUM") as xpose_pool,
            tc.tile_pool(name="upsum", bufs=2, space="PSUM") as upsum_pool,
            tc.tile_pool(name="u2psum", bufs=1, space="PSUM") as u2psum_pool,
        ):
            ident = consts.tile([128, 128], f32)
            make_identity(nc, ident[:])
            w1t = consts.tile([D, D], f32)
            w2t = consts.tile([D, O], f32)
            vA = consts.tile([128, HB], f32)
            vB = consts.tile([128, FD - HB], f32)
            nc.sync.dma_start(w1t[:], w1t_ext[:])
            nc.sync.dma_start(w2t[:], w2t_ext[:])
            nc.vector.memset(vA[:], 0.0)
            nc.vector.memset(vB[:], 0.0)

            # pre-create U chunk tiles (u2 writes target chunk c+2)
            u_tiles = [u_pool.tile([128, FD * K], f32, tag="ubuf", name=f"u_c{c}")
                       for c in range(nch)]
            EP = min(nch, 2)          # epilogue chunks (v2 lags main by 2)
            LEP = EP * K
            u2ep = u2ep_pool.tile([32, LEP], f32, tag="u2ep", name="u2ep")

            for c in range(nch):
                u_t = u_tiles[c]
                if c < 2:
                    # layer-2 inputs for the first two chunks are zero
                    nc.vector.memset(u_t[0:32, BC::FD], 0.0)

                # ---- produce u1 for chunk c ----
                # chunk 0 gates the first chain: spread its x loads over three
                # DMA queues so dispatch isn't serialized on the SP sequencer.
                dma_engs = (nc.sync, nc.scalar, nc.gpsimd) if c == 0 else (nc.sync,)
                xT = xT_pool.tile([128, BC * K], f32, tag="xT")
                for j in range(4):
                    xp = xpose_pool.tile([128, 4, 128], f32, tag="xpose")
                    for i in range(4):
                        b = 4 * j + i
                        xt = xin_pool.tile([128, 128], f32, tag="xin")
                        dma_engs[b % len(dma_engs)].dma_start(
                            xt[:], x_ext[b, c * K:(c + 1) * K, :])
                        nc.tensor.transpose(xp[:, i, :], xt[:], ident[:])
                    nc.scalar.copy(xT[:, j * 512:(j + 1) * 512], xp[:])
                for j in range(4):
                    up = upsum_pool.tile([128, 512], f32, tag="upsum")
                    nc.tensor.matmul(up[:], w1t[:], xT[:, j * 512:(j + 1) * 512],
                                     start=True, stop=True)
                    # copy u1 psum -> U chunk cols {t*FD + b}, b in [4j, 4j+4)
                    dst = u_t[:].rearrange("p (t f) -> p f t", f=FD)[:, 4 * j:4 * j + 4, :]
                    src = up[:].rearrange("p (b t) -> p b t", b=4)
                    nc.scalar.copy(dst, src)

                # ---- serial chain for chunk c (two interleaved streams) ----
                w_t = w_pool.tile([128, FD * K], f32, tag="wbuf")
                for t in range(K):
                    slA = slice(t * FD, t * FD + HB)
                    slB = slice(t * FD + HB, t * FD + FD)
                    nc.vector.tensor_tensor(out=w_t[:, slA], in0=vA[:],
                                            in1=u_t[:, slA], op=Alu.add)
                    nc.vector.tensor_tensor(out=w_t[:, slB], in0=vB[:],
                                            in1=u_t[:, slB], op=Alu.add)
                    nc.vector.scalar_tensor_tensor(out=vA[:], in0=w_t[:, slA],
                                                   scalar=1.0, in1=w_t[:, slA],
                                                   op0=Alu.is_lt, op1=Alu.mult)
                    nc.vector.scalar_tensor_tensor(out=vB[:], in0=w_t[:, slB],
                                                   scalar=1.0, in1=w_t[:, slB],
                                                   op0=Alu.is_lt, op1=Alu.mult)

                # ---- spikes s1 -> g (b-major [128, b*K+t]) ----
                g_t = g_pool.tile([128, BC * K], f32, tag="gbuf")
                g3 = g_t[:].rearrange("p (b t) -> p b t", b=BC)
                w3 = w_t[:].rearrange("p (t f) -> p f t", f=FD)[:, 0:BC, :]
                nc.gpsimd.tensor_scalar(g3, w3, 1.0, None, op0=Alu.is_ge)

                # ---- layer-2 matmuls: u2 psum [2(o), b*K+t]; then DMA remap to
                # col-16 partitions p = 16*o + b (linear element orders match).
                u2p = u2psum_pool.tile([2, BC * K], f32, tag="u2psum")
                for j in range(4):
                    nc.tensor.matmul(u2p[:, j * 512:(j + 1) * 512], w2t[:],
                                     g_t[:, j * 512:(j + 1) * 512], start=True, stop=True)
                u2s = u2sb_pool.tile([2, BC * K], f32, tag="u2sb")
                nc.scalar.copy(u2s[:], u2p[:])
                u2c = u2c_pool.tile([32, K], f32, tag="u2c")
                nc.sync.dma_start(u2c[:], u2s[:])
                if c + 2 < nch:
                    nc.scalar.copy(u_tiles[c + 2][0:32, BC::FD], u2c[:])
                else:
                    e = c + EP - nch
                    nc.scalar.copy(u2ep[:, e * K:(e + 1) * K], u2c[:])

                # ---- s2 output for time-chunk tau = c - 2 ----
                if c >= 2:
                    tau = c - 2
                    s2 = s2_pool.tile([32, K], f32, tag="s2st")
                    nc.gpsimd.tensor_scalar(s2[:], w_t[0:32, BC::FD], 1.0, None,
                                            op0=Alu.is_ge)
                    nc.sync.dma_start(out_ext[:, :, tau * K:(tau + 1) * K], s2[:])

            # ---- epilogue: layer-2 for the last EP chunks. [32,1]-operand ops
            # are effectively free on the engine (cost-model scalar exemption);
            # the loop runs at SEQ-decode rate (~140ns/step).
            for e in range(EP):
                tau = nch - EP + e
                wep = wep_pool.tile([32, K], f32, tag="wep")
                for t in range(K):
                    nc.vector.tensor_tensor(out=wep[:, t:t + 1],
                                            in0=vB[0:32, HB_V2:HB_V2 + 1],
                                            in1=u2ep[:, e * K + t:e * K + t + 1],
                                            op=Alu.add)
                    nc.vector.scalar_tensor_tensor(out=vB[0:32, HB_V2:HB_V2 + 1],
                                                   in0=wep[:, t:t + 1], scalar=1.0,
                                                   in1=wep[:, t:t + 1],
                                                   op0=Alu.is_lt, op1=Alu.mult)
                s2 = s2_pool.tile([32, K], f32, tag="s2st")
                nc.gpsimd.tensor_scalar(s2[:], wep[:], 1.0, None, op0=Alu.is_ge)
                nc.sync.dma_start(out_ext[:, :, tau * K:(tau + 1) * K], s2[:])

    nc.compile()
    return nc


# v2 column inside the B-stream state tile (global col 16 -> local col 8)
HB_V2 = FD - HB - 1

_program_cache = {}


def kernel(x, W1, W2):
    x = np.ascontiguousarray(np.asarray(x, dtype=np.float32))
    W1 = np.asarray(W1, dtype=np.float32)
    W2 = np.asarray(W2, dtype=np.float32)
    t_steps = x.shape[1]
    if t_steps not in _program_cache:
        _program_cache[t_steps] = build_program(t_steps)
    nc = _program_cache[t_steps]

    w1t = np.ascontiguousarray(W1.T)            # [d_in, d_out]
    w2t = np.ascontiguousarray(W2.T)            # [d_in, 2]
    in_maps = [
        {"x": np.ascontiguousarray(x[i * BC:(i + 1) * BC]), "w1t": w1t, "w2t": w2t}
        for i in range(NCORES)
    ]
    res = run_bass_kernel_spmd(nc, in_maps, list(range(NCORES)))
    # device layout is [O, BC, T]; full output is [B, T, O]
    outs = [np.transpose(np.asarray(res.results[i]["out"]), (1, 2, 0))
            for i in range(NCORES)]
    return np.ascontiguousarray(np.concatenate(outs, axis=0)).astype(np.float32)


# revision 11
# speedup vs baseline: 1.0176x; 1.0176x over previous
"""Trainium2 Bass kernel for BeatDetectionRSNN2 (2-layer integrate-and-fire RSNN).

Reference semantics (per time step t):
    v1 += x_t @ W1.T ; s1 = (v1 >= 1); v1 *= (1 - s1)
    v2 += s1 @ W2.T  ; s2 = (v2 >= 1); v2 *= (1 - s2)
    out[:, t, :] = s2

Sharding: data-parallel over batch across 8 cores (16 batch rows each),
weights replicated, time recurrence local per core.

Per-core plan (all shapes hardcoded: B_c=16, T=4096, D=128, O=2):
  - PE computes u1 = x @ W1.T ahead of time in K-step chunks
    (x tiles [t,d] are PE-transposed to [d,t]; one big matmul per chunk).
  - The serial recurrence runs on the vector engine over a [128, 17] state:
    cols 0..15 = v1 laid out [d, b], col 16 = v2 for the 32 (b,o) pairs on
    partitions 0..31 (layer-2 fused into the same instructions, consuming
    u2 produced 2 chunks behind). The state is split into two independent
    streams (A: batches 0-7; B: batches 8-15 + the v2 column) whose
    dependent instruction pairs are interleaved, hiding most of the
    store-visibility stall between dependent ops on the engine:
        I1: w = v + u          (tensor_tensor add)
        I2: v = (w < 1) * w    (scalar_tensor_tensor)
  - Spikes s1 = (w >= 1) are extracted per chunk (gpsimd) and fed to PE
    for u2 = s1 @ W2.T; s2 comes from col 16 of w (extracted on gpsimd).
"""
import sys
import numpy as np

if '/opt/trn_rl_repo' not in sys.path:
    sys.path.insert(0, '/opt/trn_rl_repo')

import concourse.bacc as bacc
import concourse.tile as tile
import concourse.mybir as mybir
from concourse.masks import make_identity
from concourse.bass_utils import run_bass_kernel_spmd

f32 = mybir.dt.float32
Alu = mybir.AluOpType

B, T, D, O = 128, 4096, 128, 2
NCORES = 8
BC = B // NCORES          # 16 batch rows per core
K = 128                   # chunk (time steps)
NC = T // K               # 32 chunks
FD = BC + 1               # 17 chain columns
HB = 8                    # batches in stream A (stream B: 8 batches + v2 col)


def build_program(t_steps=T):
    nch = t_steps // K
    nc = bacc.Bacc("TRN2", target_bir_lowering=False)
    x_ext = nc.declare_dram_parameter("x", [BC, t_steps, D], f32, isOutput=False)
    w1t_ext = nc.declare_dram_parameter("w1t", [D, D], f32, isOutput=False)
    w2t_ext = nc.declare_dram_parameter("w2t", [D, O], f32, isOutput=False)
    # output stored (o, b, t) so the per-chunk DMA from [32, K] staging
    # (partition p = 16*o + b) is contiguous; host transposes to [b, t, o].
    out_ext = nc.declare_dram_parameter("out", [O, BC, t_steps], f32, isOutput=True)

    with tile.TileContext(nc) as tc:
        with (
            tc.tile_pool(name="consts", bufs=1) as consts,
            tc.tile_pool(name="xin", bufs=6) as xin_pool,
            tc.tile_pool(name="xT", bufs=2) as xT_pool,
            tc.tile_pool(name="ubuf", bufs=4) as u_pool,
            tc.tile_pool(name="wbuf", bufs=3) as w_pool,
            tc.tile_pool(name="gbuf", bufs=2) as g_pool,
            tc.tile_pool(name="s2st", bufs=3) as s2_pool,
            tc.tile_pool(name="u2ep", bufs=2) as u2ep_pool,
            tc.tile_pool(name="u2sb", bufs=2) as u2sb_pool,
            tc.tile_pool(name="u2c", bufs=3) as u2c_pool,
            tc.tile_pool(name="wep", bufs=2) as wep_pool,
            tc.tile_pool(name="xpose", bufs=2, space="PSUM") as xpose_pool,
            tc.tile_pool(name="upsum", bufs=2, space="PSUM") as upsum_pool,
            tc.tile_pool(name="u2psum", bufs=1, space="PSUM") as u2psum_pool,
        ):
            ident = consts.tile([128, 128], f32)
            make_identity(nc, ident[:])
            w1t = consts.tile([D, D], f32)
            w2t = consts.tile([D, O], f32)
            vA = consts.tile([128, HB], f32)
            vB = consts.tile([128, FD - HB], f32)
            # weight loads off the SP queue: its head is the critical path for
            # chunk-0 x loads
            nc.gpsimd.dma_start(w1t[:], w1t_ext[:])
            nc.gpsimd.dma_start(w2t[:], w2t_ext[:])
            nc.vector.memset(vA[:], 0.0)
            nc.vector.memset(vB[:], 0.0)

            # pre-create U chunk tiles (u2 writes target chunk c+2)
            u_tiles = [u_pool.tile([128, FD * K], f32, tag="ubuf", name=f"u_c{c}")
                       for c in range(nch)]
            EP = min(nch, 2)          # epilogue chunks (v2 lags main by 2)
            LEP = EP * K
            u2ep = u2ep_pool.tile([32, LEP], f32, tag="u2ep", name="u2ep")

            for c in range(nch):
                u_t = u_tiles[c]
                if c < 2:
                    # layer-2 inputs for the first two chunks are zero
                    nc.vector.memset(u_t[0:32, BC::FD], 0.0)

                # ---- produce u1 for chunk c ----
                # chunk 0 gates the first chain: spread its x loads over three
                # DMA queues so dispatch isn't serialized on the SP sequencer.
                dma_engs = (nc.sync, nc.scalar, nc.gpsimd) if c == 0 else (nc.sync,)
                xT = xT_pool.tile([128, BC * K], f32, tag="xT")
                for j in range(4):
                    xp = xpose_pool.tile([128, 4, 128], f32, tag="xpose")
                    for i in range(4):
                        b = 4 * j + i
                        xt = xin_pool.tile([128, 128], f32, tag="xin")
                        dma_engs[b % len(dma_engs)].dma_start(
                            xt[:], x_ext[b, c * K:(c + 1) * K, :])
                        nc.tensor.transpose(xp[:, i, :], xt[:], ident[:])
                    nc.scalar.copy(xT[:, j * 512:(j + 1) * 512], xp[:])
                for j in range(4):
                    up = upsum_pool.tile([128, 512], f32, tag="upsum")
                    nc.tensor.matmul(up[:], w1t[:], xT[:, j * 512:(j + 1) * 512],
                                     start=True, stop=True)
                    # copy u1 psum -> U chunk cols {t*FD + b}, b in [4j, 4j+4)
                    dst = u_t[:].rearrange("p (t f) -> p f t", f=FD)[:, 4 * j:4 * j + 4, :]
                    src = up[:].rearrange("p (b t) -> p b t", b=4)
                    nc.scalar.copy(dst, src)

                # ---- serial chain for chunk c (two interleaved streams) ----
                w_t = w_pool.tile([128, FD * K], f32, tag="wbuf")
                for t in range(K):
                    slA = slice(t * FD, t * FD + HB)
                    slB = slice(t * FD + HB, t * FD + FD)
                    nc.vector.tensor_tensor(out=w_t[:, slA], in0=vA[:],
                                            in1=u_t[:, slA], op=Alu.add)
                    nc.vector.tensor_tensor(out=w_t[:, slB], in0=vB[:],
                                            in1=u_t[:, slB], op=Alu.add)
                    nc.vector.scalar_tensor_tensor(out=vA[:], in0=w_t[:, slA],
                                                   scalar=1.0, in1=w_t[:, slA],
                                                   op0=Alu.is_lt, op1=Alu.mult)
                    nc.vector.scalar_tensor_tensor(out=vB[:], in0=w_t[:, slB],
                                                   scalar=1.0, in1=w_t[:, slB],
                                                   op0=Alu.is_lt, op1=Alu.mult)

                # ---- spikes s1 -> g (b-major [128, b*K+t]) ----
                g_t = g_pool.tile([128, BC * K], f32, tag="gbuf")
                g3 = g_t[:].rearrange("p (b t) -> p b t", b=BC)
                w3 = w_t[:].rearrange("p (t f) -> p f t", f=FD)[:, 0:BC, :]
                nc.gpsimd.tensor_scalar(g3, w3, 1.0, None, op0=Alu.is_ge)

                # ---- layer-2 matmuls: u2 psum [2(o), b*K+t]; then DMA remap to
                # col-16 partitions p = 16*o + b (linear element orders match).
                u2p = u2psum_pool.tile([2, BC * K], f32, tag="u2psum")
                for j in range(4):
                    nc.tensor.matmul(u2p[:, j * 512:(j + 1) * 512], w2t[:],
                                     g_t[:, j * 512:(j + 1) * 512], start=True, stop=True)
                u2s = u2sb_pool.tile([2, BC * K], f32, tag="u2sb")
                nc.scalar.copy(u2s[:], u2p[:])
                u2c = u2c_pool.tile([32, K], f32, tag="u2c")
                nc.sync.dma_start(u2c[:], u2s[:])
                if c + 2 < nch:
                    nc.scalar.copy(u_tiles[c + 2][0:32, BC::FD], u2c[:])
                else:
                    e = c + EP - nch
                    nc.scalar.copy(u2ep[:, e * K:(e + 1) * K], u2c[:])

                # ---- s2 output for time-chunk tau = c - 2 ----
                if c >= 2:
                    tau = c - 2
                    s2 = s2_pool.tile([32, K], f32, tag="s2st")
                    nc.gpsimd.tensor_scalar(s2[:], w_t[0:32, BC::FD], 1.0, None,
                                            op0=Alu.is_ge)
                    nc.sync.dma_start(out_ext[:, :, tau * K:(tau + 1) * K], s2[:])

            # ---- epilogue: layer-2 for the last EP chunks. [32,1]-operand ops
            # are effectively free on the engine (cost-model scalar exemption);
            # the loop runs at SEQ-decode rate (~140ns/step).
            for e in range(EP):
                tau = nch - EP + e
                wep = wep_pool.tile([32, K], f32, tag="wep")
                for t in range(K):
                    nc.vector.tensor_tensor(out=wep[:, t:t + 1],
                                            in0=vB[0:32, HB_V2:HB_V2 + 1],
                                            in1=u2ep[:, e * K + t:e * K + t + 1],
                                            op=Alu.add)
                    nc.vector.scalar_tensor_tensor(out=vB[0:32, HB_V2:HB_V2 + 1],
                                                   in0=wep[:, t:t + 1], scalar=1.0,
                                                   in1=wep[:, t:t + 1],
                                                   op0=Alu.is_lt, op1=Alu.mult)
                s2 = s2_pool.tile([32, K], f32, tag="s2st")
                nc.gpsimd.tensor_scalar(s2[:], wep[:], 1.0, None, op0=Alu.is_ge)
                nc.sync.dma_start(out_ext[:, :, tau * K:(tau + 1) * K], s2[:])

    nc.compile()
    return nc


# v2 column inside the B-stream state tile (global col 16 -> local col 8)
HB_V2 = FD - HB - 1

_program_cache = {}


def kernel(x, W1, W2):
    x = np.ascontiguousarray(np.asarray(x, dtype=np.float32))
    W1 = np.asarray(W1, dtype=np.float32)
    W2 = np.asarray(W2, dtype=np.float32)
    t_steps = x.shape[1]
    if t_steps not in _program_cache:
        _program_cache[t_steps] = build_program(t_steps)
    nc = _program_cache[t_steps]

    w1t = np.ascontiguousarray(W1.T)            # [d_in, d_out]
    w2t = np.ascontiguousarray(W2.T)            # [d_in, 2]
    in_maps = [
        {"x": np.ascontiguousarray(x[i * BC:(i + 1) * BC]), "w1t": w1t, "w2t": w2t}
        for i in range(NCORES)
    ]
    res = run_bass_kernel_spmd(nc, in_maps, list(range(NCORES)))
    # device layout is [O, BC, T]; full output is [B, T, O]
    outs = [np.transpose(np.asarray(res.results[i]["out"]), (1, 2, 0))
            for i in range(NCORES)]
    return np.ascontiguousarray(np.concatenate(outs, axis=0)).astype(np.float32)


# revision 16
# speedup vs baseline: 1.1263x; 1.1068x over previous
"""Trainium2 Bass kernel for BeatDetectionRSNN2 (2-layer integrate-and-fire RSNN).

Reference semantics (per time step t):
    v1 += x_t @ W1.T ; s1 = (v1 >= 1); v1 *= (1 - s1)
    v2 += s1 @ W2.T  ; s2 = (v2 >= 1); v2 *= (1 - s2)
    out[:, t, :] = s2

Sharding: data-parallel over batch across 8 cores (16 batch rows each),
weights replicated, time recurrence local per core.

Per-core plan (all shapes hardcoded: B_c=16, T=4096, D=128, O=2):
  - PE computes u1 = x @ W1.T ahead of time in K-step chunks
    (x tiles [t,d] are PE-transposed to [d,t]; one big matmul per chunk).
  - The serial recurrence runs on the vector engine over a [128, 17] state:
    cols 0..15 = v1 laid out [d, b], col 16 = v2 for the 32 (b,o) pairs on
    partitions 0..31 (layer-2 fused into the same instructions, consuming
    u2 produced 2 chunks behind). The state is split into two independent
    streams (A: batches 0-7; B: batches 8-15 + the v2 column) whose
    dependent instruction pairs are interleaved, hiding most of the
    store-visibility stall between dependent ops on the engine:
        I1: w = v + u          (tensor_tensor add)
        I2: v = (w < 1) * w    (scalar_tensor_tensor)
  - Spikes s1 = (w >= 1) are extracted per chunk (gpsimd) and fed to PE
    for u2 = s1 @ W2.T; s2 comes from col 16 of w (extracted on gpsimd).
"""
import sys
import numpy as np

if '/opt/trn_rl_repo' not in sys.path:
    sys.path.insert(0, '/opt/trn_rl_repo')

import concourse.bacc as bacc
import concourse.tile as tile
import concourse.mybir as mybir
from concourse.masks import make_identity
from concourse.bass_utils import run_bass_kernel_spmd

f32 = mybir.dt.float32
Alu = mybir.AluOpType

B, T, D, O = 128, 4096, 128, 2
NCORES = 8
BC = B // NCORES          # 16 batch rows per core
K = 128                   # chunk (time steps)
NC = T // K               # 32 chunks
FD = BC + 1               # 17 chain columns
HB = 8                    # batches in stream A (stream B: 8 batches + v2 col)


def build_program(t_steps=T):
    nch = t_steps // K
    nc = bacc.Bacc("TRN2", target_bir_lowering=False)
    x_ext = nc.declare_dram_parameter("x", [BC, t_steps, D], f32, isOutput=False)
    w1t_ext = nc.declare_dram_parameter("w1t", [D, D], f32, isOutput=False)
    w2t_ext = nc.declare_dram_parameter("w2t", [D, O], f32, isOutput=False)
    # output stored (o, b, t) so the per-chunk DMA from [32, K] staging
    # (partition p = 16*o + b) is contiguous; host transposes to [b, t, o].
    out_ext = nc.declare_dram_parameter("out", [O, BC, t_steps], f32, isOutput=True)

    with tile.TileContext(nc) as tc:
        with (
            tc.tile_pool(name="consts", bufs=1) as consts,
            tc.tile_pool(name="xin", bufs=6) as xin_pool,
            tc.tile_pool(name="xT", bufs=2) as xT_pool,
            tc.tile_pool(name="ubuf", bufs=4) as u_pool,
            tc.tile_pool(name="wbuf", bufs=3) as w_pool,
            tc.tile_pool(name="gbuf", bufs=2) as g_pool,
            tc.tile_pool(name="s2st", bufs=3) as s2_pool,
            tc.tile_pool(name="u2ep", bufs=2) as u2ep_pool,
            tc.tile_pool(name="u2sb", bufs=2) as u2sb_pool,
            tc.tile_pool(name="u2c", bufs=3) as u2c_pool,
            tc.tile_pool(name="wep", bufs=2) as wep_pool,
            tc.tile_pool(name="xpose", bufs=2, space="PSUM") as xpose_pool,
            tc.tile_pool(name="upsum", bufs=2, space="PSUM") as upsum_pool,
            tc.tile_pool(name="u2psum", bufs=1, space="PSUM") as u2psum_pool,
        ):
            ident = consts.tile([128, 128], f32)
            make_identity(nc, ident[:])
            w1t = consts.tile([D, D], f32)
            w2t = consts.tile([D, O], f32)
            vA = consts.tile([128, HB], f32)
            vB = consts.tile([128, FD - HB], f32)
            # weight loads off the SP queue: its head is the critical path for
            # chunk-0 x loads
            nc.gpsimd.dma_start(w1t[:], w1t_ext[:])
            nc.gpsimd.dma_start(w2t[:], w2t_ext[:])
            nc.vector.memset(vA[:], 0.0)
            nc.vector.memset(vB[:], 0.0)

            # pre-create U chunk tiles (u2 writes target chunk c+2)
            u_tiles = [u_pool.tile([128, FD * K], f32, tag="ubuf", name=f"u_c{c}")
                       for c in range(nch)]
            # v2 (col 16) lags the main chain by D steps: just enough for the
            # u2 path (extract+matmul+copy) of chunk c to land in chunk c+1's
            # tail slots before the chain reads them. Shorter lag = shorter
            # serial epilogue (D steps instead of 2K).
            D2 = 176
            DK = D2 - K               # 32: u2 slots crossing into chunk c+2
            EPS = min(D2, t_steps)    # epilogue steps
            u2ep = u2ep_pool.tile([32, D2], f32, tag="u2ep", name="u2ep")

            for c in range(nch):
                u_t = u_tiles[c]
                if c == 0:
                    # v2-steps < 0: zeros
                    nc.vector.memset(u_t[0:32, BC::FD], 0.0)
                elif c == 1:
                    nc.vector.memset(u_t[0:32, BC:DK * FD:FD], 0.0)

                # ---- produce u1 for chunk c ----
                # chunk 0 gates the first chain: spread its x loads over three
                # DMA queues so dispatch isn't serialized on the SP sequencer.
                dma_engs = (nc.sync, nc.scalar, nc.gpsimd) if c == 0 else (nc.sync,)
                xT = xT_pool.tile([128, BC * K], f32, tag="xT")
                for j in range(4):
                    xp = xpose_pool.tile([128, 4, 128], f32, tag="xpose")
                    for i in range(4):
                        b = 4 * j + i
                        xt = xin_pool.tile([128, 128], f32, tag="xin")
                        dma_engs[b % len(dma_engs)].dma_start(
                            xt[:], x_ext[b, c * K:(c + 1) * K, :])
                        nc.tensor.transpose(xp[:, i, :], xt[:], ident[:])
                    nc.scalar.copy(xT[:, j * 512:(j + 1) * 512], xp[:])
                for j in range(4):
                    up = upsum_pool.tile([128, 512], f32, tag="upsum")
                    nc.tensor.matmul(up[:], w1t[:], xT[:, j * 512:(j + 1) * 512],
                                     start=True, stop=True)
                    # copy u1 psum -> U chunk cols {t*FD + b}, b in [4j, 4j+4)
                    dst = u_t[:].rearrange("p (t f) -> p f t", f=FD)[:, 4 * j:4 * j + 4, :]
                    src = up[:].rearrange("p (b t) -> p b t", b=4)
                    nc.scalar.copy(dst, src)

                # ---- serial chain for chunk c (two interleaved streams) ----
                w_t = w_pool.tile([128, FD * K], f32, tag="wbuf")
                for t in range(K):
                    slA = slice(t * FD, t * FD + HB)
                    slB = slice(t * FD + HB, t * FD + FD)
                    nc.vector.tensor_tensor(out=w_t[:, slA], in0=vA[:],
                                            in1=u_t[:, slA], op=Alu.add)
                    nc.vector.tensor_tensor(out=w_t[:, slB], in0=vB[:],
                                            in1=u_t[:, slB], op=Alu.add)
                    nc.vector.scalar_tensor_tensor(out=vA[:], in0=w_t[:, slA],
                                                   scalar=1.0, in1=w_t[:, slA],
                                                   op0=Alu.is_lt, op1=Alu.mult)
                    nc.vector.scalar_tensor_tensor(out=vB[:], in0=w_t[:, slB],
                                                   scalar=1.0, in1=w_t[:, slB],
                                                   op0=Alu.is_lt, op1=Alu.mult)

                # ---- spikes s1 -> g (b-major [128, b*K+t]) ----
                g_t = g_pool.tile([128, BC * K], f32, tag="gbuf")
                g3 = g_t[:].rearrange("p (b t) -> p b t", b=BC)
                w3 = w_t[:].rearrange("p (t f) -> p f t", f=FD)[:, 0:BC, :]
                nc.gpsimd.tensor_scalar(g3, w3, 1.0, None, op0=Alu.is_ge)

                # ---- layer-2 matmuls: u2 psum [2(o), b*K+t]; then DMA remap to
                # col-16 partitions p = 16*o + b (linear element orders match).
                u2p = u2psum_pool.tile([2, BC * K], f32, tag="u2psum")
                for j in range(4):
                    nc.tensor.matmul(u2p[:, j * 512:(j + 1) * 512], w2t[:],
                                     g_t[:, j * 512:(j + 1) * 512], start=True, stop=True)
                u2s = u2sb_pool.tile([2, BC * K], f32, tag="u2sb")
                nc.scalar.copy(u2s[:], u2p[:])
                u2c = u2c_pool.tile([32, K], f32, tag="u2c")
                nc.sync.dma_start(u2c[:], u2s[:])
                # scatter u2(c): steps [0, K-DK) -> chunk c+1 col-16 slots
                # [DK, K); steps [K-DK, K) -> chunk c+2 slots [0, DK). Chunks
                # past the end route into the epilogue staging tile at index
                # s + DK - (nch-1-c)*K.
                if c + 1 < nch:
                    nc.scalar.copy(u_tiles[c + 1][0:32, DK * FD + BC::FD],
                                   u2c[:, 0:K - DK])
                else:
                    off = DK - (nch - 1 - c) * K
                    nc.scalar.copy(u2ep[:, off:off + K - DK], u2c[:, 0:K - DK])
                if c + 2 < nch:
                    nc.scalar.copy(u_tiles[c + 2][0:32, BC:DK * FD:FD],
                                   u2c[:, K - DK:K])
                else:
                    off = DK - (nch - 1 - c) * K + (K - DK)
                    nc.scalar.copy(u2ep[:, off:off + DK], u2c[:, K - DK:K])

                # ---- s2 output: col 16 of chunk c covers v2-steps
                # [c*K - D2, c*K + K - D2) ----
                if c * K >= D2:
                    s2 = s2_pool.tile([32, K], f32, tag="s2st")
                    nc.gpsimd.tensor_scalar(s2[:], w_t[0:32, BC::FD], 1.0, None,
                                            op0=Alu.is_ge)
                    nc.sync.dma_start(out_ext[:, :, c * K - D2:c * K + K - D2],
                                      s2[:])
                elif c * K + K - D2 > 0:
                    n_val = c * K + K - D2
                    s2 = s2_pool.tile([32, K], f32, tag="s2st")
                    nc.gpsimd.tensor_scalar(s2[:, 0:n_val],
                                            w_t[0:32, (K - n_val) * FD + BC::FD],
                                            1.0, None, op0=Alu.is_ge)
                    nc.sync.dma_start(out_ext[:, :, 0:n_val], s2[:, 0:n_val])

            # ---- epilogue: layer-2 for the last EPS steps. [32,1]-operand ops
            # are effectively free on the engine (cost-model scalar exemption);
            # the loop runs at SEQ-decode rate (~140ns/step).
            wep = wep_pool.tile([32, D2], f32, tag="wep")
            for t in range(D2 - EPS, D2):
                nc.vector.tensor_tensor(out=wep[:, t:t + 1],
                                        in0=vB[0:32, HB_V2:HB_V2 + 1],
                                        in1=u2ep[:, t:t + 1], op=Alu.add)
                nc.vector.scalar_tensor_tensor(out=vB[0:32, HB_V2:HB_V2 + 1],
                                               in0=wep[:, t:t + 1], scalar=1.0,
                                               in1=wep[:, t:t + 1],
                                               op0=Alu.is_lt, op1=Alu.mult)
            s2e = s2_pool.tile([32, D2], f32, tag="s2ep", name="s2e")
            nc.gpsimd.tensor_scalar(s2e[:, D2 - EPS:D2], wep[:, D2 - EPS:D2],
                                    1.0, None, op0=Alu.is_ge)
            nc.sync.dma_start(out_ext[:, :, t_steps - EPS:t_steps],
                              s2e[:, D2 - EPS:D2])

    nc.compile()
    return nc


# v2 column inside the B-stream state tile (global col 16 -> local col 8)
HB_V2 = FD - HB - 1

_program_cache = {}


def kernel(x, W1, W2):
    x = np.ascontiguousarray(np.asarray(x, dtype=np.float32))
    W1 = np.asarray(W1, dtype=np.float32)
    W2 = np.asarray(W2, dtype=np.float32)
    t_steps = x.shape[1]
    if t_steps not in _program_cache:
        _program_cache[t_steps] = build_program(t_steps)
    nc = _program_cache[t_steps]

    w1t = np.ascontiguousarray(W1.T)            # [d_in, d_out]
    w2t = np.ascontiguousarray(W2.T)            # [d_in, 2]
    in_maps = [
        {"x": np.ascontiguousarray(x[i * BC:(i + 1) * BC]), "w1t": w1t, "w2t": w2t}
        for i in range(NCORES)
    ]
    res = run_bass_kernel_spmd(nc, in_maps, list(range(NCORES)))
    # device layout is [O, BC, T]; full output is [B, T, O]
    outs = [np.transpose(np.asarray(res.results[i]["out"]), (1, 2, 0))
            for i in range(NCORES)]
    return np.ascontiguousarray(np.concatenate(outs, axis=0)).astype(np.float32)


# revision 18
# speedup vs baseline: 1.1293x; 1.0027x over previous
"""Trainium2 Bass kernel for BeatDetectionRSNN2 (2-layer integrate-and-fire RSNN).

Reference semantics (per time step t):
    v1 += x_t @ W1.T ; s1 = (v1 >= 1); v1 *= (1 - s1)
    v2 += s1 @ W2.T  ; s2 = (v2 >= 1); v2 *= (1 - s2)
    out[:, t, :] = s2

Sharding: data-parallel over batch across 8 cores (16 batch rows each),
weights replicated, time recurrence local per core.

Per-core plan (all shapes hardcoded: B_c=16, T=4096, D=128, O=2):
  - PE computes u1 = x @ W1.T ahead of time in K-step chunks
    (x tiles [t,d] are PE-transposed to [d,t]; one big matmul per chunk).
  - The serial recurrence runs on the vector engine over a [128, 17] state:
    cols 0..15 = v1 laid out [d, b], col 16 = v2 for the 32 (b,o) pairs on
    partitions 0..31 (layer-2 fused into the same instructions, consuming
    u2 produced 2 chunks behind). The state is split into two independent
    streams (A: batches 0-7; B: batches 8-15 + the v2 column) whose
    dependent instruction pairs are interleaved, hiding most of the
    store-visibility stall between dependent ops on the engine:
        I1: w = v + u          (tensor_tensor add)
        I2: v = (w < 1) * w    (scalar_tensor_tensor)
  - Spikes s1 = (w >= 1) are extracted per chunk (gpsimd) and fed to PE
    for u2 = s1 @ W2.T; s2 comes from col 16 of w (extracted on gpsimd).
"""
import sys
import numpy as np

if '/opt/trn_rl_repo' not in sys.path:
    sys.path.insert(0, '/opt/trn_rl_repo')

import concourse.bacc as bacc
import concourse.tile as tile
import concourse.mybir as mybir
from concourse.masks import make_identity
from concourse.bass_utils import run_bass_kernel_spmd

f32 = mybir.dt.float32
Alu = mybir.AluOpType

B, T, D, O = 128, 4096, 128, 2
NCORES = 8
BC = B // NCORES          # 16 batch rows per core
K = 128                   # chunk (time steps)
NC = T // K               # 32 chunks
FD = BC + 1               # 17 chain columns
HB = 8                    # batches in stream A (stream B: 8 batches + v2 col)


def build_program(t_steps=T):
    nch = t_steps // K
    nc = bacc.Bacc("TRN2", target_bir_lowering=False)
    x_ext = nc.declare_dram_parameter("x", [BC, t_steps, D], f32, isOutput=False)
    w1t_ext = nc.declare_dram_parameter("w1t", [D, D], f32, isOutput=False)
    w2t_ext = nc.declare_dram_parameter("w2t", [D, O], f32, isOutput=False)
    # output stored (o, b, t) so the per-chunk DMA from [32, K] staging
    # (partition p = 16*o + b) is contiguous; host transposes to [b, t, o].
    out_ext = nc.declare_dram_parameter("out", [O, BC, t_steps], f32, isOutput=True)

    with tile.TileContext(nc) as tc:
        with (
            tc.tile_pool(name="consts", bufs=1) as consts,
            tc.tile_pool(name="xin", bufs=2) as xin_pool,
            tc.tile_pool(name="xT", bufs=2) as xT_pool,
            tc.tile_pool(name="ubuf", bufs=4) as u_pool,
            tc.tile_pool(name="wbuf", bufs=3) as w_pool,
            tc.tile_pool(name="gbuf", bufs=2) as g_pool,
            tc.tile_pool(name="s2st", bufs=3) as s2_pool,
            tc.tile_pool(name="u2ep", bufs=2) as u2ep_pool,
            tc.tile_pool(name="u2sb", bufs=2) as u2sb_pool,
            tc.tile_pool(name="u2c", bufs=3) as u2c_pool,
            tc.tile_pool(name="wep", bufs=2) as wep_pool,
            tc.tile_pool(name="xpose", bufs=2, space="PSUM") as xpose_pool,
            tc.tile_pool(name="upsum", bufs=2, space="PSUM") as upsum_pool,
            tc.tile_pool(name="u2psum", bufs=1, space="PSUM") as u2psum_pool,
        ):
            ident = consts.tile([128, 128], f32)
            make_identity(nc, ident[:])
            w1t = consts.tile([D, D], f32)
            w2t = consts.tile([D, O], f32)
            vA = consts.tile([128, HB], f32)
            vB = consts.tile([128, FD - HB], f32)
            # weight loads off the SP queue: its head is the critical path for
            # chunk-0 x loads
            nc.gpsimd.dma_start(w1t[:], w1t_ext[:])
            nc.gpsimd.dma_start(w2t[:], w2t_ext[:])
            nc.vector.memset(vA[:], 0.0)
            nc.vector.memset(vB[:], 0.0)

            # pre-create U chunk tiles (u2 writes target chunk c+2)
            u_tiles = [u_pool.tile([128, FD * K], f32, tag="ubuf", name=f"u_c{c}")
                       for c in range(nch)]
            # v2 (col 16) lags the main chain by D steps: just enough for the
            # u2 path (extract+matmul+copy) of chunk c to land in chunk c+1's
            # tail slots before the chain reads them. Shorter lag = shorter
            # serial epilogue (D steps instead of 2K).
            D2 = 176
            DK = D2 - K               # 32: u2 slots crossing into chunk c+2
            EPS = min(D2, t_steps)    # epilogue steps
            u2ep = u2ep_pool.tile([32, D2], f32, tag="u2ep", name="u2ep")

            for c in range(nch):
                u_t = u_tiles[c]
                if c == 0:
                    # v2-steps < 0: zeros
                    nc.vector.memset(u_t[0:32, BC::FD], 0.0)
                elif c == 1:
                    nc.vector.memset(u_t[0:32, BC:DK * FD:FD], 0.0)

                # ---- produce u1 for chunk c ----
                # one 3D DMA per half-chunk loads x for 8 batches at once
                # ([t partitions, (b, d) free], d contiguous) — far fewer SP
                # dispatches than per-batch tile loads. Chunk 0 gates the first
                # chain: put its two loads on different queues.
                xt_all = xin_pool.tile([128, BC, 128], f32, tag="xin")
                for h in range(2):
                    eng = nc.scalar if (c == 0 and h == 1) else nc.sync
                    eng.dma_start(xt_all[:, 8 * h:8 * h + 8, :],
                                  x_ext[8 * h:8 * h + 8, c * K:(c + 1) * K, :]
                                  .rearrange("b t d -> t b d"))
                xT = xT_pool.tile([128, BC * K], f32, tag="xT")
                for j in range(4):
                    xp = xpose_pool.tile([128, 4, 128], f32, tag="xpose")
                    for i in range(4):
                        b = 4 * j + i
                        nc.tensor.transpose(xp[:, i, :], xt_all[:, b, :], ident[:])
                    nc.scalar.copy(xT[:, j * 512:(j + 1) * 512], xp[:])
                for j in range(4):
                    up = upsum_pool.tile([128, 512], f32, tag="upsum")
                    nc.tensor.matmul(up[:], w1t[:], xT[:, j * 512:(j + 1) * 512],
                                     start=True, stop=True)
                    # copy u1 psum -> U chunk cols {t*FD + b}, b in [4j, 4j+4)
                    dst = u_t[:].rearrange("p (t f) -> p f t", f=FD)[:, 4 * j:4 * j + 4, :]
                    src = up[:].rearrange("p (b t) -> p b t", b=4)
                    nc.scalar.copy(dst, src)

                # ---- serial chain for chunk c (two interleaved streams) ----
                w_t = w_pool.tile([128, FD * K], f32, tag="wbuf")
                for t in range(K):
                    slA = slice(t * FD, t * FD + HB)
                    slB = slice(t * FD + HB, t * FD + FD)
                    nc.vector.tensor_tensor(out=w_t[:, slA], in0=vA[:],
                                            in1=u_t[:, slA], op=Alu.add)
                    nc.vector.tensor_tensor(out=w_t[:, slB], in0=vB[:],
                                            in1=u_t[:, slB], op=Alu.add)
                    nc.vector.scalar_tensor_tensor(out=vA[:], in0=w_t[:, slA],
                                                   scalar=1.0, in1=w_t[:, slA],
                                                   op0=Alu.is_lt, op1=Alu.mult)
                    nc.vector.scalar_tensor_tensor(out=vB[:], in0=w_t[:, slB],
                                                   scalar=1.0, in1=w_t[:, slB],
                                                   op0=Alu.is_lt, op1=Alu.mult)

                # ---- spikes s1 -> g (b-major [128, b*K+t]) ----
                g_t = g_pool.tile([128, BC * K], f32, tag="gbuf")
                g3 = g_t[:].rearrange("p (b t) -> p b t", b=BC)
                w3 = w_t[:].rearrange("p (t f) -> p f t", f=FD)[:, 0:BC, :]
                nc.gpsimd.tensor_scalar(g3, w3, 1.0, None, op0=Alu.is_ge)

                # ---- layer-2 matmuls: u2 psum [2(o), b*K+t]; then DMA remap to
                # col-16 partitions p = 16*o + b (linear element orders match).
                u2p = u2psum_pool.tile([2, BC * K], f32, tag="u2psum")
                for j in range(4):
                    nc.tensor.matmul(u2p[:, j * 512:(j + 1) * 512], w2t[:],
                                     g_t[:, j * 512:(j + 1) * 512], start=True, stop=True)
                u2s = u2sb_pool.tile([2, BC * K], f32, tag="u2sb")
                nc.scalar.copy(u2s[:], u2p[:])
                u2c = u2c_pool.tile([32, K], f32, tag="u2c")
                nc.sync.dma_start(u2c[:], u2s[:])
                # scatter u2(c): steps [0, K-DK) -> chunk c+1 col-16 slots
                # [DK, K); steps [K-DK, K) -> chunk c+2 slots [0, DK). Chunks
                # past the end route into the epilogue staging tile at index
                # s + DK - (nch-1-c)*K.
                if c + 1 < nch:
                    nc.scalar.copy(u_tiles[c + 1][0:32, DK * FD + BC::FD],
                                   u2c[:, 0:K - DK])
                else:
                    off = DK - (nch - 1 - c) * K
                    nc.scalar.copy(u2ep[:, off:off + K - DK], u2c[:, 0:K - DK])
                if c + 2 < nch:
                    nc.scalar.copy(u_tiles[c + 2][0:32, BC:DK * FD:FD],
                                   u2c[:, K - DK:K])
                else:
                    off = DK - (nch - 1 - c) * K + (K - DK)
                    nc.scalar.copy(u2ep[:, off:off + DK], u2c[:, K - DK:K])

                # ---- s2 output: col 16 of chunk c covers v2-steps
                # [c*K - D2, c*K + K - D2) ----
                if c * K >= D2:
                    s2 = s2_pool.tile([32, K], f32, tag="s2st")
                    nc.gpsimd.tensor_scalar(s2[:], w_t[0:32, BC::FD], 1.0, None,
                                            op0=Alu.is_ge)
                    nc.sync.dma_start(out_ext[:, :, c * K - D2:c * K + K - D2],
                                      s2[:])
                elif c * K + K - D2 > 0:
                    n_val = c * K + K - D2
                    s2 = s2_pool.tile([32, K], f32, tag="s2st")
                    nc.gpsimd.tensor_scalar(s2[:, 0:n_val],
                                            w_t[0:32, (K - n_val) * FD + BC::FD],
                                            1.0, None, op0=Alu.is_ge)
                    nc.sync.dma_start(out_ext[:, :, 0:n_val], s2[:, 0:n_val])

            # ---- epilogue: layer-2 for the last EPS steps. [32,1]-operand ops
            # are effectively free on the engine (cost-model scalar exemption);
            # the loop runs at SEQ-decode rate (~140ns/step).
            wep = wep_pool.tile([32, D2], f32, tag="wep")
            for t in range(D2 - EPS, D2):
                nc.vector.tensor_tensor(out=wep[:, t:t + 1],
                                        in0=vB[0:32, HB_V2:HB_V2 + 1],
                                        in1=u2ep[:, t:t + 1], op=Alu.add)
                nc.vector.scalar_tensor_tensor(out=vB[0:32, HB_V2:HB_V2 + 1],
                                               in0=wep[:, t:t + 1], scalar=1.0,
                                               in1=wep[:, t:t + 1],
                                               op0=Alu.is_lt, op1=Alu.mult)
            s2e = s2_pool.tile([32, D2], f32, tag="s2ep", name="s2e")
            nc.gpsimd.tensor_scalar(s2e[:, D2 - EPS:D2], wep[:, D2 - EPS:D2],
                                    1.0, None, op0=Alu.is_ge)
            nc.sync.dma_start(out_ext[:, :, t_steps - EPS:t_steps],
                              s2e[:, D2 - EPS:D2])

    nc.compile()
    return nc


# v2 column inside the B-stream state tile (global col 16 -> local col 8)
HB_V2 = FD - HB - 1

_program_cache = {}


def kernel(x, W1, W2):
    x = np.ascontiguousarray(np.asarray(x, dtype=np.float32))
    W1 = np.asarray(W1, dtype=np.float32)
    W2 = np.asarray(W2, dtype=np.float32)
    t_steps = x.shape[1]
    if t_steps not in _program_cache:
        _program_cache[t_steps] = build_program(t_steps)
    nc = _program_cache[t_steps]

    w1t = np.ascontiguousarray(W1.T)            # [d_in, d_out]
    w2t = np.ascontiguousarray(W2.T)            # [d_in, 2]
    in_maps = [
        {"x": np.ascontiguousarray(x[i * BC:(i + 1) * BC]), "w1t": w1t, "w2t": w2t}
        for i in range(NCORES)
    ]
    res = run_bass_kernel_spmd(nc, in_maps, list(range(NCORES)))
    # device layout is [O, BC, T]; full output is [B, T, O]
    outs = [np.transpose(np.asarray(res.results[i]["out"]), (1, 2, 0))
            for i in range(NCORES)]
    return np.ascontiguousarray(np.concatenate(outs, axis=0)).astype(np.float32)


# revision 21
# speedup vs baseline: 1.1574x; 1.0249x over previous
"""Trainium2 Bass kernel for BeatDetectionRSNN2 (2-layer integrate-and-fire RSNN).

Reference semantics (per time step t):
    v1 += x_t @ W1.T ; s1 = (v1 >= 1); v1 *= (1 - s1)
    v2 += s1 @ W2.T  ; s2 = (v2 >= 1); v2 *= (1 - s2)
    out[:, t, :] = s2

Sharding: data-parallel over batch across 8 cores (16 batch rows each),
weights replicated, time recurrence local per core.

Per-core plan (all shapes hardcoded: B_c=16, T=4096, D=128, O=2):
  - PE computes u1 = x @ W1.T ahead of time in K-step chunks
    (x tiles [t,d] are PE-transposed to [d,t]; one big matmul per chunk).
  - The serial recurrence runs on the vector engine over a [128, 17] state:
    cols 0..15 = v1 laid out [d, b], col 16 = v2 for the 32 (b,o) pairs on
    partitions 0..31 (layer-2 fused into the same instructions, consuming
    u2 produced 2 chunks behind). The state is split into two independent
    streams (A: batches 0-7; B: batches 8-15 + the v2 column) whose
    dependent instruction pairs are interleaved, hiding most of the
    store-visibility stall between dependent ops on the engine:
        I1: w = v + u          (tensor_tensor add)
        I2: v = (w < 1) * w    (scalar_tensor_tensor)
  - Spikes s1 = (w >= 1) are extracted per chunk (gpsimd) and fed to PE
    for u2 = s1 @ W2.T; s2 comes from col 16 of w (extracted on gpsimd).
"""
import sys
import numpy as np

if '/opt/trn_rl_repo' not in sys.path:
    sys.path.insert(0, '/opt/trn_rl_repo')

import concourse.bacc as bacc
import concourse.tile as tile
import concourse.mybir as mybir
from concourse.masks import make_identity
from concourse.bass_utils import run_bass_kernel_spmd

f32 = mybir.dt.float32
Alu = mybir.AluOpType

B, T, D, O = 128, 4096, 128, 2
NCORES = 8
BC = B // NCORES          # 16 batch rows per core
K = 128                   # chunk (time steps)
NC = T // K               # 32 chunks
FD = BC + 1               # 17 chain columns
HB = 8                    # batches in stream A (stream B: 8 batches + v2 col)


def build_program(t_steps=T):
    nch = t_steps // K
    nc = bacc.Bacc("TRN2", target_bir_lowering=False)
    x_ext = nc.declare_dram_parameter("x", [BC, t_steps, D], f32, isOutput=False)
    w1t_ext = nc.declare_dram_parameter("w1t", [D, D], f32, isOutput=False)
    w2t_ext = nc.declare_dram_parameter("w2t", [D, O], f32, isOutput=False)
    # output stored (o, b, t) so the per-chunk DMA from [32, K] staging
    # (partition p = 16*o + b) is contiguous; host transposes to [b, t, o].
    out_ext = nc.declare_dram_parameter("out", [O, BC, t_steps], f32, isOutput=True)

    with tile.TileContext(nc) as tc:
        with (
            tc.tile_pool(name="consts", bufs=1) as consts,
            tc.tile_pool(name="xin", bufs=2) as xin_pool,
            tc.tile_pool(name="xT", bufs=2) as xT_pool,
            tc.tile_pool(name="ubuf", bufs=4) as u_pool,
            tc.tile_pool(name="wbuf", bufs=3) as w_pool,
            tc.tile_pool(name="gbuf", bufs=2) as g_pool,
            tc.tile_pool(name="s2st", bufs=3) as s2_pool,
            tc.tile_pool(name="u2ep", bufs=2) as u2ep_pool,
            tc.tile_pool(name="u2sb", bufs=2) as u2sb_pool,
            tc.tile_pool(name="u2c", bufs=3) as u2c_pool,
            tc.tile_pool(name="wep", bufs=2) as wep_pool,
            tc.tile_pool(name="xpose", bufs=2, space="PSUM") as xpose_pool,
            tc.tile_pool(name="upsum", bufs=2, space="PSUM") as upsum_pool,
            tc.tile_pool(name="u2psum", bufs=1, space="PSUM") as u2psum_pool,
        ):
            ident = consts.tile([128, 128], f32)
            make_identity(nc, ident[:])
            w1t = consts.tile([D, D], f32)
            w2t = consts.tile([D, O], f32)
            vA = consts.tile([128, HB], f32)
            vB = consts.tile([128, FD - HB], f32)
            # weight loads off the SP queue: its head is the critical path for
            # chunk-0 x loads
            nc.gpsimd.dma_start(w1t[:], w1t_ext[:])
            nc.gpsimd.dma_start(w2t[:], w2t_ext[:])
            nc.vector.memset(vA[:], 0.0)
            nc.vector.memset(vB[:], 0.0)

            # pre-create U chunk tiles (u2 writes target chunk c+2)
            u_tiles = [u_pool.tile([128, FD * K], f32, tag="ubuf", name=f"u_c{c}")
                       for c in range(nch)]
            # v2 (col 16) lags the main chain by D steps: just enough for the
            # u2 path (extract+matmul+copy) of chunk c to land in chunk c+1's
            # tail slots before the chain reads them. Shorter lag = shorter
            # serial epilogue (D steps instead of 2K).
            D2 = 160
            DK = D2 - K               # 32: u2 slots crossing into chunk c+2
            EPS = min(D2, t_steps)    # epilogue steps
            u2ep = u2ep_pool.tile([32, D2], f32, tag="u2ep", name="u2ep")

            for c in range(nch):
                u_t = u_tiles[c]
                if c == 0:
                    # v2-steps < 0: zeros
                    nc.vector.memset(u_t[0:32, BC::FD], 0.0)
                elif c == 1:
                    nc.vector.memset(u_t[0:32, BC:DK * FD:FD], 0.0)

                # ---- produce u1 for chunk c ----
                # one 3D DMA per half-chunk loads x for 8 batches at once
                # ([t partitions, (b, d) free], d contiguous) — far fewer SP
                # dispatches than per-batch tile loads. Chunk 0 gates the first
                # chain: put its two loads on different queues.
                xt_all = xin_pool.tile([128, BC, 128], f32, tag="xin")
                for h in range(2):
                    eng = nc.scalar if (c == 0 and h == 1) else nc.sync
                    eng.dma_start(xt_all[:, 8 * h:8 * h + 8, :],
                                  x_ext[8 * h:8 * h + 8, c * K:(c + 1) * K, :]
                                  .rearrange("b t d -> t b d"))
                xT = xT_pool.tile([128, BC * K], f32, tag="xT")
                for j in range(4):
                    xp = xpose_pool.tile([128, 4, 128], f32, tag="xpose")
                    for i in range(4):
                        b = 4 * j + i
                        nc.tensor.transpose(xp[:, i, :], xt_all[:, b, :], ident[:])
                    nc.scalar.copy(xT[:, j * 512:(j + 1) * 512], xp[:])
                for j in range(4):
                    up = upsum_pool.tile([128, 512], f32, tag="upsum")
                    nc.tensor.matmul(up[:], w1t[:], xT[:, j * 512:(j + 1) * 512],
                                     start=True, stop=True)
                    # copy u1 psum -> U chunk cols {t*FD + b}, b in [4j, 4j+4)
                    dst = u_t[:].rearrange("p (t f) -> p f t", f=FD)[:, 4 * j:4 * j + 4, :]
                    src = up[:].rearrange("p (b t) -> p b t", b=4)
                    nc.scalar.copy(dst, src)

                # ---- serial chain for chunk c (two interleaved streams) ----
                w_t = w_pool.tile([128, FD * K], f32, tag="wbuf")
                for t in range(K):
                    slA = slice(t * FD, t * FD + HB)
                    slB = slice(t * FD + HB, t * FD + FD)
                    nc.vector.tensor_tensor(out=w_t[:, slA], in0=vA[:],
                                            in1=u_t[:, slA], op=Alu.add)
                    nc.vector.tensor_tensor(out=w_t[:, slB], in0=vB[:],
                                            in1=u_t[:, slB], op=Alu.add)
                    nc.vector.scalar_tensor_tensor(out=vA[:], in0=w_t[:, slA],
                                                   scalar=1.0, in1=w_t[:, slA],
                                                   op0=Alu.is_lt, op1=Alu.mult)
                    nc.vector.scalar_tensor_tensor(out=vB[:], in0=w_t[:, slB],
                                                   scalar=1.0, in1=w_t[:, slB],
                                                   op0=Alu.is_lt, op1=Alu.mult)

                # ---- spikes s1 -> g in 4 batch-groups, each feeding its u2
                # matmul immediately; u2 psum [2(o), b*K+t] DMA-remapped
                # straight from PSUM to partitions p = 16*o + b (linear
                # element orders match). Shorter path = shorter v2 lag D2.
                g_t = g_pool.tile([128, BC * K], f32, tag="gbuf")
                g3 = g_t[:].rearrange("p (b t) -> p b t", b=BC)
                w3 = w_t[:].rearrange("p (t f) -> p f t", f=FD)
                u2p = u2psum_pool.tile([2, BC * K], f32, tag="u2psum")
                u2s = u2sb_pool.tile([2, BC * K], f32, tag="u2sb")
                for j in range(4):
                    nc.gpsimd.tensor_scalar(g3[:, 4 * j:4 * j + 4, :],
                                            w3[:, 4 * j:4 * j + 4, :],
                                            1.0, None, op0=Alu.is_ge)
                    nc.tensor.matmul(u2p[:, j * 512:(j + 1) * 512], w2t[:],
                                     g_t[:, j * 512:(j + 1) * 512], start=True, stop=True)
                    nc.scalar.copy(u2s[:, j * 512:(j + 1) * 512],
                                   u2p[:, j * 512:(j + 1) * 512])
                u2c = u2c_pool.tile([32, K], f32, tag="u2c")
                nc.sync.dma_start(u2c[:], u2s[:])
                # scatter u2(c): steps [0, K-DK) -> chunk c+1 col-16 slots
                # [DK, K); steps [K-DK, K) -> chunk c+2 slots [0, DK). Chunks
                # past the end route into the epilogue staging tile at index
                # s + DK - (nch-1-c)*K.
                if c + 1 < nch:
                    nc.scalar.copy(u_tiles[c + 1][0:32, DK * FD + BC::FD],
                                   u2c[:, 0:K - DK])
                else:
                    off = DK - (nch - 1 - c) * K
                    nc.scalar.copy(u2ep[:, off:off + K - DK], u2c[:, 0:K - DK])
                if c + 2 < nch:
                    nc.scalar.copy(u_tiles[c + 2][0:32, BC:DK * FD:FD],
                                   u2c[:, K - DK:K])
                else:
                    off = DK - (nch - 1 - c) * K + (K - DK)
                    nc.scalar.copy(u2ep[:, off:off + DK], u2c[:, K - DK:K])

                # ---- s2 output: col 16 of chunk c covers v2-steps
                # [c*K - D2, c*K + K - D2) ----
                if c * K >= D2:
                    s2 = s2_pool.tile([32, K], f32, tag="s2st")
                    nc.gpsimd.tensor_scalar(s2[:], w_t[0:32, BC::FD], 1.0, None,
                                            op0=Alu.is_ge)
                    nc.sync.dma_start(out_ext[:, :, c * K - D2:c * K + K - D2],
                                      s2[:])
                elif c * K + K - D2 > 0:
                    n_val = c * K + K - D2
                    s2 = s2_pool.tile([32, K], f32, tag="s2st")
                    nc.gpsimd.tensor_scalar(s2[:, 0:n_val],
                                            w_t[0:32, (K - n_val) * FD + BC::FD],
                                            1.0, None, op0=Alu.is_ge)
                    nc.sync.dma_start(out_ext[:, :, 0:n_val], s2[:, 0:n_val])

            # ---- epilogue: layer-2 for the last EPS steps. [32,1]-operand ops
            # are effectively free on the engine (cost-model scalar exemption);
            # the loop runs at SEQ-decode rate (~140ns/step).
            wep = wep_pool.tile([32, D2], f32, tag="wep")
            for t in range(D2 - EPS, D2):
                nc.vector.tensor_tensor(out=wep[:, t:t + 1],
                                        in0=vB[0:32, HB_V2:HB_V2 + 1],
                                        in1=u2ep[:, t:t + 1], op=Alu.add)
                nc.vector.scalar_tensor_tensor(out=vB[0:32, HB_V2:HB_V2 + 1],
                                               in0=wep[:, t:t + 1], scalar=1.0,
                                               in1=wep[:, t:t + 1],
                                               op0=Alu.is_lt, op1=Alu.mult)
            s2e = s2_pool.tile([32, D2], f32, tag="s2ep", name="s2e")
            nc.gpsimd.tensor_scalar(s2e[:, D2 - EPS:D2], wep[:, D2 - EPS:D2],
                                    1.0, None, op0=Alu.is_ge)
            nc.sync.dma_start(out_ext[:, :, t_steps - EPS:t_steps],
                              s2e[:, D2 - EPS:D2])

    nc.compile()
    return nc


# v2 column inside the B-stream state tile (global col 16 -> local col 8)
HB_V2 = FD - HB - 1

_program_cache = {}


def kernel(x, W1, W2):
    x = np.ascontiguousarray(np.asarray(x, dtype=np.float32))
    W1 = np.asarray(W1, dtype=np.float32)
    W2 = np.asarray(W2, dtype=np.float32)
    t_steps = x.shape[1]
    if t_steps not in _program_cache:
        _program_cache[t_steps] = build_program(t_steps)
    nc = _program_cache[t_steps]

    w1t = np.ascontiguousarray(W1.T)            # [d_in, d_out]
    w2t = np.ascontiguousarray(W2.T)            # [d_in, 2]
    in_maps = [
        {"x": np.ascontiguousarray(x[i * BC:(i + 1) * BC]), "w1t": w1t, "w2t": w2t}
        for i in range(NCORES)
    ]
    res = run_bass_kernel_spmd(nc, in_maps, list(range(NCORES)))
    # device layout is [O, BC, T]; full output is [B, T, O]
    outs = [np.transpose(np.asarray(res.results[i]["out"]), (1, 2, 0))
            for i in range(NCORES)]
    return np.ascontiguousarray(np.concatenate(outs, axis=0)).astype(np.float32)


# revision 28
# speedup vs baseline: 1.1606x; 1.0027x over previous
"""Trainium2 Bass kernel for BeatDetectionRSNN2 (2-layer integrate-and-fire RSNN).

Reference semantics (per time step t):
    v1 += x_t @ W1.T ; s1 = (v1 >= 1); v1 *= (1 - s1)
    v2 += s1 @ W2.T  ; s2 = (v2 >= 1); v2 *= (1 - s2)
    out[:, t, :] = s2

Sharding: data-parallel over batch across 8 cores (16 batch rows each),
weights replicated, time recurrence local per core.

Per-core plan (all shapes hardcoded: B_c=16, T=4096, D=128, O=2):
  - PE computes u1 = x @ W1.T ahead of time in K-step chunks
    (x tiles [t,d] are PE-transposed to [d,t]; one big matmul per chunk).
  - The serial recurrence runs on the vector engine over a [128, 17] state:
    cols 0..15 = v1 laid out [d, b], col 16 = v2 for the 32 (b,o) pairs on
    partitions 0..31 (layer-2 fused into the same instructions, consuming
    u2 produced 2 chunks behind). The state is split into two independent
    streams (A: batches 0-7; B: batches 8-15 + the v2 column) whose
    dependent instruction pairs are interleaved, hiding most of the
    store-visibility stall between dependent ops on the engine:
        I1: w = v + u          (tensor_tensor add)
        I2: v = (w < 1) * w    (scalar_tensor_tensor)
  - Spikes s1 = (w >= 1) are extracted per chunk (gpsimd) and fed to PE
    for u2 = s1 @ W2.T; s2 comes from col 16 of w (extracted on gpsimd).
"""
import sys
import numpy as np

if '/opt/trn_rl_repo' not in sys.path:
    sys.path.insert(0, '/opt/trn_rl_repo')

import concourse.bacc as bacc
import concourse.tile as tile
import concourse.mybir as mybir
from concourse.masks import make_identity
from concourse.bass_utils import run_bass_kernel_spmd

f32 = mybir.dt.float32
Alu = mybir.AluOpType

B, T, D, O = 128, 4096, 128, 2
NCORES = 8
BC = B // NCORES          # 16 batch rows per core
K = 128                   # chunk (time steps)
NC = T // K               # 32 chunks
FD = BC + 1               # 17 chain columns
HB = 8                    # batches in stream A (stream B: 8 batches + v2 col)


def build_program(t_steps=T):
    nch = t_steps // K
    nc = bacc.Bacc("TRN2", target_bir_lowering=False)
    x_ext = nc.declare_dram_parameter("x", [BC, t_steps, D], f32, isOutput=False)
    w1t_ext = nc.declare_dram_parameter("w1t", [D, D], f32, isOutput=False)
    w2t_ext = nc.declare_dram_parameter("w2t", [D, O], f32, isOutput=False)
    # output stored (o, b, t) so the per-chunk DMA from [32, K] staging
    # (partition p = 16*o + b) is contiguous; host transposes to [b, t, o].
    out_ext = nc.declare_dram_parameter("out", [O, BC, t_steps], f32, isOutput=True)

    with tile.TileContext(nc) as tc:
        with (
            tc.tile_pool(name="consts", bufs=1) as consts,
            tc.tile_pool(name="xin", bufs=2) as xin_pool,
            tc.tile_pool(name="xT", bufs=2) as xT_pool,
            tc.tile_pool(name="ubuf", bufs=4) as u_pool,
            tc.tile_pool(name="wbuf", bufs=3) as w_pool,
            tc.tile_pool(name="gbuf", bufs=2) as g_pool,
            tc.tile_pool(name="s2st", bufs=3) as s2_pool,
            tc.tile_pool(name="u2ep", bufs=2) as u2ep_pool,
            tc.tile_pool(name="u2sb", bufs=2) as u2sb_pool,
            tc.tile_pool(name="u2c", bufs=3) as u2c_pool,
            tc.tile_pool(name="wep", bufs=2) as wep_pool,
            tc.tile_pool(name="xpose", bufs=2, space="PSUM") as xpose_pool,
            tc.tile_pool(name="upsum", bufs=2, space="PSUM") as upsum_pool,
            tc.tile_pool(name="u2psum", bufs=1, space="PSUM") as u2psum_pool,
        ):
            ident = consts.tile([128, 128], f32)
            make_identity(nc, ident[:])
            w1t = consts.tile([D, D], f32)
            w2t = consts.tile([D, O], f32)
            vA = consts.tile([128, HB], f32)
            vB = consts.tile([128, FD - HB], f32)
            # weight loads off the SP queue: its head is the critical path for
            # chunk-0 x loads
            nc.gpsimd.dma_start(w1t[:], w1t_ext[:])
            nc.gpsimd.dma_start(w2t[:], w2t_ext[:])
            nc.vector.memset(vA[:], 0.0)
            nc.vector.memset(vB[:], 0.0)

            # pre-create U chunk tiles (u2 writes target chunk c+2)
            u_tiles = [u_pool.tile([128, FD * K], f32, tag="ubuf", name=f"u_c{c}")
                       for c in range(nch)]
            # v2 (col 16) lags the main chain by D steps: just enough for the
            # u2 path (extract+matmul+copy) of chunk c to land in chunk c+1's
            # tail slots before the chain reads them. Shorter lag = shorter
            # serial epilogue (D steps instead of 2K).
            D2 = 160
            DK = D2 - K               # 32: u2 slots crossing into chunk c+2
            EPS = min(D2, t_steps)    # epilogue steps
            u2ep = u2ep_pool.tile([32, D2], f32, tag="u2ep", name="u2ep")

            for c in range(nch):
                u_t = u_tiles[c]
                if c == 0:
                    # v2-steps < 0: zeros
                    nc.vector.memset(u_t[0:32, BC::FD], 0.0)
                elif c == 1:
                    nc.vector.memset(u_t[0:32, BC:DK * FD:FD], 0.0)

                # ---- produce u1 for chunk c ----
                # one 3D DMA per half-chunk loads x for 8 batches at once
                # ([t partitions, (b, d) free], d contiguous) — far fewer SP
                # dispatches than per-batch tile loads. Chunk 0 gates the first
                # chain: put its two loads on different queues.
                xt_all = xin_pool.tile([128, BC, 128], f32, tag="xin")
                for h in range(2):
                    eng = nc.scalar if (c == 0 and h == 1) else nc.sync
                    eng.dma_start(xt_all[:, 8 * h:8 * h + 8, :],
                                  x_ext[8 * h:8 * h + 8, c * K:(c + 1) * K, :]
                                  .rearrange("b t d -> t b d"))
                # xT is t-major (cols = t*16 + b) so each u1 matmul covers a
                # 32-step block for ALL batches — the chain's first steps then
                # wait on only the first matmul+copy, not all four.
                xT = xT_pool.tile([128, BC * K], f32, tag="xT")
                for j in range(4):
                    xp = xpose_pool.tile([128, 4, 128], f32, tag="xpose")
                    for i in range(4):
                        b = 4 * j + i
                        nc.tensor.transpose(xp[:, i, :], xt_all[:, b, :], ident[:])
                    dstx = xT[:].rearrange("p (t b) -> p b t", b=BC)[:, 4 * j:4 * j + 4, :]
                    nc.scalar.copy(dstx, xp[:])
                for j in range(4):
                    up = upsum_pool.tile([128, 512], f32, tag="upsum")
                    nc.tensor.matmul(up[:], w1t[:], xT[:, j * 512:(j + 1) * 512],
                                     start=True, stop=True)
                    # copy u1 psum (t-major) -> U chunk steps [32j, 32j+32)
                    dst = u_t[:].rearrange("p (t f) -> p t f", f=FD)[:, 32 * j:32 * j + 32, 0:BC]
                    src = up[:].rearrange("p (t b) -> p t b", b=BC)
                    nc.scalar.copy(dst, src)

                # ---- serial chain for chunk c (two interleaved streams) ----
                w_t = w_pool.tile([128, FD * K], f32, tag="wbuf")
                for t in range(K):
                    slA = slice(t * FD, t * FD + HB)
                    slB = slice(t * FD + HB, t * FD + FD)
                    nc.vector.tensor_tensor(out=w_t[:, slA], in0=vA[:],
                                            in1=u_t[:, slA], op=Alu.add)
                    nc.vector.tensor_tensor(out=w_t[:, slB], in0=vB[:],
                                            in1=u_t[:, slB], op=Alu.add)
                    nc.vector.scalar_tensor_tensor(out=vA[:], in0=w_t[:, slA],
                                                   scalar=1.0, in1=w_t[:, slA],
                                                   op0=Alu.is_lt, op1=Alu.mult)
                    nc.vector.scalar_tensor_tensor(out=vB[:], in0=w_t[:, slB],
                                                   scalar=1.0, in1=w_t[:, slB],
                                                   op0=Alu.is_lt, op1=Alu.mult)

                # ---- spikes s1 -> g in 4 batch-groups, each feeding its u2
                # matmul immediately; u2 psum [2(o), b*K+t] DMA-remapped
                # straight from PSUM to partitions p = 16*o + b (linear
                # element orders match). Shorter path = shorter v2 lag D2.
                g_t = g_pool.tile([128, BC * K], f32, tag="gbuf")
                g3 = g_t[:].rearrange("p (b t) -> p b t", b=BC)
                w3 = w_t[:].rearrange("p (t f) -> p f t", f=FD)
                u2p = u2psum_pool.tile([2, BC * K], f32, tag="u2psum")
                u2s = u2sb_pool.tile([2, BC * K], f32, tag="u2sb")
                for j in range(4):
                    nc.gpsimd.tensor_scalar(g3[:, 4 * j:4 * j + 4, :],
                                            w3[:, 4 * j:4 * j + 4, :],
                                            1.0, None, op0=Alu.is_ge)
                    nc.tensor.matmul(u2p[:, j * 512:(j + 1) * 512], w2t[:],
                                     g_t[:, j * 512:(j + 1) * 512], start=True, stop=True)
                    nc.scalar.copy(u2s[:, j * 512:(j + 1) * 512],
                                   u2p[:, j * 512:(j + 1) * 512])
                u2c = u2c_pool.tile([32, K], f32, tag="u2c")
                nc.sync.dma_start(u2c[:], u2s[:])
                # scatter u2(c): steps [0, K-DK) -> chunk c+1 col-16 slots
                # [DK, K); steps [K-DK, K) -> chunk c+2 slots [0, DK). Chunks
                # past the end route into the epilogue staging tile at index
                # s + DK - (nch-1-c)*K.
                if c + 1 < nch:
                    nc.scalar.copy(u_tiles[c + 1][0:32, DK * FD + BC::FD],
                                   u2c[:, 0:K - DK])
                else:
                    off = DK - (nch - 1 - c) * K
                    nc.scalar.copy(u2ep[:, off:off + K - DK], u2c[:, 0:K - DK])
                if c + 2 < nch:
                    nc.scalar.copy(u_tiles[c + 2][0:32, BC:DK * FD:FD],
                                   u2c[:, K - DK:K])
                else:
                    off = DK - (nch - 1 - c) * K + (K - DK)
                    nc.scalar.copy(u2ep[:, off:off + DK], u2c[:, K - DK:K])

                # ---- s2 output: col 16 of chunk c covers v2-steps
                # [c*K - D2, c*K + K - D2) ----
                if c * K >= D2:
                    s2 = s2_pool.tile([32, K], f32, tag="s2st")
                    nc.gpsimd.tensor_scalar(s2[:], w_t[0:32, BC::FD], 1.0, None,
                                            op0=Alu.is_ge)
                    nc.sync.dma_start(out_ext[:, :, c * K - D2:c * K + K - D2],
                                      s2[:])
                elif c * K + K - D2 > 0:
                    n_val = c * K + K - D2
                    s2 = s2_pool.tile([32, K], f32, tag="s2st")
                    nc.gpsimd.tensor_scalar(s2[:, 0:n_val],
                                            w_t[0:32, (K - n_val) * FD + BC::FD],
                                            1.0, None, op0=Alu.is_ge)
                    nc.sync.dma_start(out_ext[:, :, 0:n_val], s2[:, 0:n_val])

            # ---- epilogue: layer-2 for the last EPS steps. [32,1]-operand ops
            # are effectively free on the engine (cost-model scalar exemption);
            # the loop runs at SEQ-decode rate (~140ns/step).
            wep = wep_pool.tile([32, D2], f32, tag="wep")
            for t in range(D2 - EPS, D2):
                nc.vector.tensor_tensor(out=wep[:, t:t + 1],
                                        in0=vB[0:32, HB_V2:HB_V2 + 1],
                                        in1=u2ep[:, t:t + 1], op=Alu.add)
                nc.vector.scalar_tensor_tensor(out=vB[0:32, HB_V2:HB_V2 + 1],
                                               in0=wep[:, t:t + 1], scalar=1.0,
                                               in1=wep[:, t:t + 1],
                                               op0=Alu.is_lt, op1=Alu.mult)
            s2e = s2_pool.tile([32, D2], f32, tag="s2ep", name="s2e")
            nc.gpsimd.tensor_scalar(s2e[:, D2 - EPS:D2], wep[:, D2 - EPS:D2],
                                    1.0, None, op0=Alu.is_ge)
            nc.sync.dma_start(out_ext[:, :, t_steps - EPS:t_steps],
                              s2e[:, D2 - EPS:D2])

    nc.compile()
    return nc


# v2 column inside the B-stream state tile (global col 16 -> local col 8)
HB_V2 = FD - HB - 1

_program_cache = {}


def kernel(x, W1, W2):
    x = np.ascontiguousarray(np.asarray(x, dtype=np.float32))
    W1 = np.asarray(W1, dtype=np.float32)
    W2 = np.asarray(W2, dtype=np.float32)
    t_steps = x.shape[1]
    if t_steps not in _program_cache:
        _program_cache[t_steps] = build_program(t_steps)
    nc = _program_cache[t_steps]

    w1t = np.ascontiguousarray(W1.T)            # [d_in, d_out]
    w2t = np.ascontiguousarray(W2.T)            # [d_in, 2]
    in_maps = [
        {"x": np.ascontiguousarray(x[i * BC:(i + 1) * BC]), "w1t": w1t, "w2t": w2t}
        for i in range(NCORES)
    ]
    res = run_bass_kernel_spmd(nc, in_maps, list(range(NCORES)))
    # device layout is [O, BC, T]; full output is [B, T, O]
    outs = [np.transpose(np.asarray(res.results[i]["out"]), (1, 2, 0))
            for i in range(NCORES)]
    return np.ascontiguousarray(np.concatenate(outs, axis=0)).astype(np.float32)
